# revision 24
# baseline (speedup 1.0000x reference)
"""GraphConv x3 + segment_max pooling on Trainium2 (8 NeuronCores, one launch).

Algorithm (per core, dst-sharded 25000 nodes):
  - 8 gather streams = src blocks of 25000 nodes; partition group j holds the
    feature table of block j ([16ch x 25000] bf16) -> no table replication.
  - Per layer, 50 chunks over the core's dst nodes (500-node windows):
    Pool:  indirect_copy gathers each stream's edges (sorted by dst) from its
           table group (<=1024 indices per instruction); a second
           indirect_copy picks node-boundary positions out of the running sum.
    DVE:   tensor_tensor_scan computes the fp32 running sum of the gathered
           bf16 edge stream; a shifted subtract turns boundary samples into
           per-(node, stream) partial sums.
    PE:    one matmul folds the 8 stream partials and the W_rel transform
           (stationary [128, Co]); a second accumulates the root term.
    ACT:   relu(psum + bias) evacuation.
  - Layers 2/3 aggregate pre-transformed features (h @ W_rel.T, <=16 ch), so
    the next layer's table is built by a small chunked matmul, bounced to HBM
    and AllGathered across the 8 cores ([16,25000] -> [128,25000] = the next
    table, exactly).
  - Per-core node tensors (root input, h1..h3) live in two [128, 25000] bf16
    buffers (rows = channels, zero-padded; SBUF charges 128 partitions either
    way), so every matmul is a plain full-height position-(0,0) matmul.
  - segment_max pooling + final linear run on host from the [8, 25000] bf16
    per-core layer-3 output (tiny download).
Any device failure falls back to a pure-numpy reference implementation.
"""
import sys
import time
import types

import numpy as np

sys.path.insert(0, "/opt/trn_rl_repo")

N = 200_000
E = 6_400_000
G = 512
NC = 8            # cores (dst shards)
NPC = 25_000      # nodes per core
NSTR = 8          # streams = src blocks
NBS = 25_000      # nodes per src block
NWIN = 50         # chunks per core per layer
WIN = 500         # dst nodes per chunk window (x4B fits one PSUM bank)
BCH = 512         # boundary slots per chunk (501 padded to mult 16)
GRP = 3_125       # nodes per fold group (= NPC / 8)

CO = (32, 16, 8)          # layer output channels
CRAW = (16, 32, 16)       # root-term contraction (padded) channels


# ---------------------------------------------------------------- host prep

def _build_streams(src, dst):
    """eidx [NC, NWIN, 128, CH//16] u16, bidx [NC, NWIN, 128, BCH//16] u16."""
    k = dst // NPC
    j = src // NBS
    dl = dst % NPC
    c = dl // WIN
    tloc = (src % NBS).astype(np.uint16)

    key = ((k * NSTR + j) * NWIN + c) * NPC + dl
    order = np.argsort(key, kind="stable")
    tloc_s = tloc[order]
    grp = (key // NPC)[order]
    dl_s = dl[order]

    ngrp = NC * NSTR * NWIN
    cnt = np.bincount(grp, minlength=ngrp)
    CH = int(16 * ((cnt.max() + 1 + 15) // 16))
    starts = np.zeros(ngrp + 1, np.int64)
    np.cumsum(cnt, out=starts[1:])
    pos = np.arange(len(src)) - starts[grp] + 1

    eidx = np.zeros((ngrp, CH), np.uint16)
    eidx[grp, pos] = tloc_s

    node_in_win = dl_s % WIN
    nkey = grp * WIN + node_in_win
    ncnt = np.bincount(nkey, minlength=ngrp * WIN).reshape(ngrp, WIN)
    P = np.zeros((ngrp, BCH), np.int64)
    np.cumsum(ncnt, axis=1, out=P[:, 1:WIN + 1])
    P[:, WIN + 1:] = P[:, WIN:WIN + 1]
    assert P.max() < CH
    bidx = P.astype(np.uint16)

    def wrap(a, L):
        a = a.reshape(NC, NSTR, NWIN, L // 16, 16)
        a = a.transpose(0, 2, 1, 4, 3).reshape(NC, NWIN, 128, L // 16)
        return np.ascontiguousarray(a)

    return wrap(eidx, CH), wrap(bidx, BCH), CH


def _make_xtab(z16, dtype):
    """z16 [N, 16] -> [128, NBS]: row 16j+c = ch c of nodes [NBS*j, NBS*(j+1))."""
    t = np.zeros((128, NBS), np.float32)
    for j in range(NSTR):
        t[16 * j:16 * j + 16, :] = z16[j * NBS:(j + 1) * NBS, :].T
    return t.astype(dtype)


def _pad128(w, cols=None):
    """w [rows_real, co] -> [128, cols or co] zero-padded."""
    cols = w.shape[1] if cols is None else cols
    out = np.zeros((128, cols), np.float32)
    out[:w.shape[0], :w.shape[1]] = w
    return out


def _np_reference(x, src, dst, batch, Ws):
    (W1r, b1, W1o, W2r, b2, W2o, W3r, b3, W3o, Wl, bl) = Ws
    def conv(h, Wr, b, Wo):
        agg = np.zeros((N, h.shape[1]), np.float32)
        np.add.at(agg, dst, h[src])
        return np.maximum(agg @ Wr.T + b + h @ Wo.T, 0.0)
    h = conv(x, W1r, b1, W1o)
    h = conv(h, W2r, b2, W2o)
    h = conv(h, W3r, b3, W3o)
    pooled = np.full((G, 8), -np.inf, np.float32)
    np.maximum.at(pooled, batch, h)
    return (pooled @ Wl.T + bl)[:, 0]


# ---------------------------------------------------------------- device

def _build_device(CH):
    import concourse.bass as bass
    import concourse.mybir as mybir
    import concourse.tile as tile
    import kutil_inline

    kutil_inline.patch_tile()
    f32, f32r, bf16, u16 = (mybir.dt.float32, mybir.dt.float32r,
                            mybir.dt.bfloat16, mybir.dt.uint16)
    nc = bass.Bass("TRN2", num_devices=NC)

    xtabB_h = nc.dram_tensor("xtabB", [128, NBS], bf16, kind="ExternalInput")
    xroot_h = nc.dram_tensor("xroot", [16, NPC], bf16, kind="ExternalInput")
    eidx_h = nc.dram_tensor("eidx", [NWIN, 128, CH // 16], u16,
                            kind="ExternalInput")
    bidx_h = nc.dram_tensor("bidx", [NWIN, 128, BCH // 16], u16,
                            kind="ExternalInput")
    tw_h = [nc.dram_tensor(f"tw{l}", [128, CO[l]], f32r, kind="ExternalInput")
            for l in range(3)]
    wroot_h = [nc.dram_tensor(f"wroot{l}", [128, CO[l]], bf16,
                              kind="ExternalInput") for l in range(3)]
    wz_h = [nc.dram_tensor(f"wz{l}", [128, 16], bf16, kind="ExternalInput")
            for l in range(2)]
    bias_h = [nc.dram_tensor(f"bias{l}", [CO[l], 1], f32, kind="ExternalInput")
              for l in range(3)]
    h3_h = nc.dram_tensor("h3", [8, NPC], bf16, kind="ExternalOutput")

    with tile.TileContext(nc) as tc:
        with tc.tile_pool(name="p", bufs=1) as pool, \
             tc.tile_pool(name="ps", bufs=3, space="PSUM") as psp, \
             tc.tile_pool(name="dram", bufs=1, space="DRAM") as dram:

            tw = []
            for l in range(3):
                t = pool.tile([128, CO[l]], f32r, name=f"tw{l}", tag=f"tw{l}")
                nc.sync.dma_start(t[:], tw_h[l][:, :])
                tw.append(t)
            wroot = []
            for l in range(3):
                t = pool.tile([128, CO[l]], bf16, name=f"wroot{l}",
                              tag=f"wroot{l}")
                nc.sync.dma_start(t[:], wroot_h[l][:, :])
                wroot.append(t)
            wz = []
            for l in range(2):
                t = pool.tile([128, 16], bf16, name=f"wz{l}", tag=f"wz{l}")
                nc.sync.dma_start(t[:], wz_h[l][:, :])
                wz.append(t)
            bias = []
            for l in range(3):
                b = pool.tile([CO[l], 1], f32, name=f"bias{l}", tag=f"bias{l}")
                nc.sync.dma_start(b[:], bias_h[l][:, :])
                bias.append(b)

            # two channel-major node buffers (rows = channels, zero padded)
            bufA = pool.tile([128, NPC], bf16)   # xroot (L1) then h2 (L2/L3)
            bufB = pool.tile([128, NPC], bf16)   # h1 (L1/L2) then h3 (L3)
            nc.vector.memset(bufA[:], 0.0)
            nc.vector.memset(bufB[:], 0.0)
            nc.sync.dma_start(bufA[0:16, :], xroot_h[:, :])
            table = pool.tile([128, NBS], bf16)

            zb = dram.tile([16, NBS], bf16)
            ccout = [dram.tile([128, NBS], bf16, addr_space="Shared",
                               name=f"ccout{i}", tag=f"ccout{i}")
                     for i in range(2)]

            zsrc = [bufA, bufB, bufA]      # root input per layer
            hdst = [(bufB, 32), (bufA, 16), (bufB, 8)]  # h output per layer

            for l in range(3):
                if l == 0:
                    nc.sync.dma_start(table[:], xtabB_h[:, :])
                else:
                    nc.sync.dma_start(table[:], ccout[l - 1][:, :])
                for c in range(NWIN):
                    cw = c * WIN
                    ei = pool.tile([128, CH // 16], u16, tag="ei", bufs=3)
                    nc.sync.dma_start(ei[:], eidx_h[c, :, :])
                    gout = pool.tile([128, CH], bf16, tag="gout", bufs=2)
                    for off in range(0, CH, 1024):
                        n = min(1024, CH - off)
                        nc.gpsimd.indirect_copy(
                            gout[:, off:off + n], table[:],
                            ei[:, off // 16:(off + n) // 16],
                            i_know_ap_gather_is_preferred=True)
                    sout = pool.tile([128, CH], f32, tag="sout", bufs=2)
                    nc.vector.tensor_tensor_scan(
                        sout[:], gout[:], gout[:], 0.0,
                        op0=mybir.AluOpType.add, op1=mybir.AluOpType.bypass)
                    bi = pool.tile([128, BCH // 16], u16, tag="bi", bufs=3)
                    nc.sync.dma_start(bi[:], bidx_h[c, :, :])
                    bout = pool.tile([128, BCH], f32, tag="bout", bufs=2)
                    nc.gpsimd.indirect_copy(
                        bout[:], sout[:], bi[:],
                        i_know_ap_gather_is_preferred=True)
                    dout = pool.tile([128, WIN], f32r, tag="dout", bufs=2)
                    nc.vector.tensor_tensor(
                        dout[:], bout[:, 1:WIN + 1], bout[:, 0:WIN],
                        op=mybir.AluOpType.subtract)
                    ps = psp.tile([CO[l], WIN], f32, tag="ps")
                    nc.tensor.matmul(
                        ps[:], tw[l][:], dout[:],
                        start=True, stop=False)
                    nc.tensor.matmul(
                        ps[:], wroot[l][:], zsrc[l][:, cw:cw + WIN],
                        start=False, stop=True)
                    htile, nch = hdst[l]
                    nc.scalar.activation(
                        htile[0:nch, cw:cw + WIN], ps[:],
                        mybir.ActivationFunctionType.Relu, bias=bias[l][:])
                if l < 2:
                    hp = bufB if l == 0 else bufA
                    for c in range(NWIN):
                        cw = c * WIN
                        psz = psp.tile([16, WIN], f32, tag="ps")
                        nc.tensor.matmul(psz[:], wz[l][:],
                                         hp[:, cw:cw + WIN],
                                         start=True, stop=True)
                        zst = pool.tile([16, WIN], bf16, tag="zst", bufs=3)
                        nc.scalar.activation(
                            zst[:], psz[:],
                            mybir.ActivationFunctionType.Copy)
                        nc.sync.dma_start(zb[:, cw:cw + WIN], zst[:])
                    nc.gpsimd.collective_compute(
                        "AllGather", mybir.AluOpType.bypass,
                        replica_groups=[list(range(NC))],
                        ins=[zb[:]], outs=[ccout[l][:]])
            nc.sync.dma_start(h3_h[:, :], bufB[0:8, :])
    return nc


# ---------------------------------------------------------------- kernel

def kernel(**inputs):
    x = np.asarray(inputs["x"], np.float32)
    ei = np.asarray(inputs["edge_index"])
    batch = np.asarray(inputs["batch"]).astype(np.int64)
    src = ei[0].astype(np.int64)
    dst = ei[1].astype(np.int64)
    Ws = tuple(np.asarray(inputs[n], np.float32) for n in
               ("W1_rel", "b1", "W1_root", "W2_rel", "b2", "W2_root",
                "W3_rel", "b3", "W3_root", "W_lin", "b_lin"))
    (W1r, b1, W1o, W2r, b2, W2o, W3r, b3, W3o, Wl, bl) = Ws
    try:
        import ml_dtypes
        from concourse.bass_utils import run_bass_kernel_spmd

        t0 = time.time()
        eidx, bidx, CH = _build_streams(src, dst)
        z16 = np.zeros((N, 16), np.float32)
        z16[:, :3] = x
        xtabB = _make_xtab(z16, ml_dtypes.bfloat16)
        tw = [np.zeros((128, CO[l]), np.float32) for l in range(3)]
        for j in range(NSTR):
            tw[0][16 * j:16 * j + 3, :] = W1r.T
            tw[1][16 * j:16 * j + 16, :] = np.eye(16, dtype=np.float32)
            tw[2][16 * j:16 * j + 8, :] = np.eye(8, dtype=np.float32)
        prep_s = time.time() - t0

        bf = ml_dtypes.bfloat16
        common = {
            "xtabB": xtabB,
            "tw0": tw[0], "tw1": tw[1], "tw2": tw[2],
            "wroot0": _pad128(W1o.T).astype(bf),
            "wroot1": _pad128(W2o.T).astype(bf),
            "wroot2": _pad128(W3o.T).astype(bf),
            "wz0": _pad128(W2r.T, 16).astype(bf),
            "wz1": _pad128(W3r.T, 16).astype(bf),
            "bias0": b1[:, None].astype(np.float32),
            "bias1": b2[:, None].astype(np.float32),
            "bias2": b3[:, None].astype(np.float32),
        }
        in_maps = []
        for k in range(NC):
            xr = np.zeros((16, NPC), np.float32)
            xr[:3, :] = x[k * NPC:(k + 1) * NPC, :].T
            in_maps.append(dict(
                common, xroot=xr.astype(bf),
                eidx=eidx[k], bidx=bidx[k]))

        t0 = time.time()
        nc = _build_device(CH)
        build_s = time.time() - t0
        t0 = time.time()
        res = run_bass_kernel_spmd(nc, in_maps, core_ids=list(range(NC)))
        exec_s = time.time() - t0
        kernel.last_hw_s = exec_s
        kernel.last_info = (prep_s, build_s, exec_s)

        h3 = np.concatenate(
            [np.asarray(res.results[k]["h3"]).astype(np.float32).T
             for k in range(NC)], axis=0)
        bounds = np.searchsorted(batch, np.arange(G))
        pooled = np.maximum.reduceat(h3, bounds, axis=0)
        empty = bounds == np.append(bounds[1:], N)
        pooled[empty] = 0.0
        out = (pooled @ Wl.T + bl)[:, 0].astype(np.float32)
        return out
    except Exception as e:  # pragma: no cover - device fallback
        import traceback
        traceback.print_exc()
        print(f"[kernel] device path failed ({type(e).__name__}: {e}); "
              f"falling back to numpy", file=sys.stderr)
        kernel.last_hw_s = -1.0
        return _np_reference(x, src, dst, batch, Ws).astype(np.float32)


# inline copy of the tile patch so kernel.py is self-contained
kutil_inline = types.ModuleType("kutil_inline")
_KUTIL_SRC = '''
import sys
sys.path.insert(0, "/opt/trn_rl_repo")
import bass_rust
import concourse.mybir as mybir
import concourse.tile as tilemod
from concourse.vector_clock import ScopedClock

MAX_WAITS = 1
_patched = False


def _split_waits(nc, ordered):
    for bb_name, insts in ordered.items():
        new_list = []
        for inst in insts:
            si = inst.sync_info
            waits = list(si.on_wait) if si is not None and si.on_wait else []
            if len(waits) > MAX_WAITS:
                keep = waits[-MAX_WAITS:]
                extra = waits[:-MAX_WAITS]
                for w in extra:
                    nop = bass_rust.InstNoOp(
                        name=f"I-{nc.next_id()}-waitnop", ins=[], outs=[]
                    )
                    nop.engine = inst.engine
                    nop.sync_info = mybir.SyncInfo(on_wait=[w], on_update=[])
                    nc.register_instruction(nop, overwrite=True)
                    new_list.append(nop)
                inst.sync_info = mybir.SyncInfo(
                    on_wait=keep,
                    on_update=list(si.on_update) if si.on_update else [],
                )
            new_list.append(inst)
        insts[:] = new_list
    return ordered


def patch_tile(verbose=False):
    global _patched
    if _patched:
        return
    _patched = True
    orig_lower = tilemod.TileContext._lower_ordered_insts

    def _lower_ordered_insts(self, ordered):
        _split_waits(self.nc, ordered)
        return orig_lower(self, ordered)

    tilemod.TileContext._lower_ordered_insts = _lower_ordered_insts

    def _drain_and_barrier(self, tick_clock, wait_clock):
        drain_inst = self.nc.sync.drain()
        wait_clock.add_sem_waits(
            drain_inst.ins, ScopedClock({None: tick_clock.global_clock})
        )
        si = drain_inst.ins.sync_info
        waits = list(si.on_wait) if si is not None and si.on_wait else []
        if len(waits) > MAX_WAITS:
            drain_inst.ins.sync_info = mybir.SyncInfo(
                on_wait=waits[:MAX_WAITS],
                on_update=list(si.on_update) if si.on_update else [],
            )
            for i in range(MAX_WAITS, len(waits), MAX_WAITS):
                d2 = self.nc.sync.drain()
                d2.ins.sync_info = mybir.SyncInfo(
                    on_wait=waits[i : i + MAX_WAITS], on_update=[]
                )
        self.nc.all_engine_barrier()
        assert self.sems is not None
        popped = self.nc._tile_sem_poison_stack.pop()
        assert popped is self._sem_poison
        self.nc.clear_and_free_semaphores(list(self.sems.allocated().values()))
        self.nc.all_engine_barrier()

    tilemod.TileContext._drain_and_barrier = _drain_and_barrier
'''
exec(_KUTIL_SRC, kutil_inline.__dict__)
sys.modules["kutil_inline"] = kutil_inline


# revision 26
# speedup vs baseline: 96.7196x; 96.7196x over previous
"""GraphConv x3 + segment_max pooling on Trainium2 (8 NeuronCores, one launch).

Algorithm (per core, dst-sharded 25000 nodes):
  - 8 gather streams = src blocks of 25000 nodes; partition group j holds the
    feature table of block j ([16ch x 25000] bf16) -> no table replication.
  - Per layer, 50 chunks over the core's dst nodes (500-node windows):
    Pool:  indirect_copy gathers each stream's edges (sorted by dst) from its
           table group (<=1024 indices per instruction); a second
           indirect_copy picks node-boundary positions out of the running sum.
    DVE:   tensor_tensor_scan computes the fp32 running sum of the gathered
           bf16 edge stream; a shifted subtract turns boundary samples into
           per-(node, stream) partial sums.
    PE:    one matmul folds the 8 stream partials and the W_rel transform
           (stationary [128, Co]); a second accumulates the root term.
    ACT:   relu(psum + bias) evacuation.
  - Layers 2/3 aggregate pre-transformed features (h @ W_rel.T, <=16 ch), so
    the next layer's table is built by a small chunked matmul, bounced to HBM
    and AllGathered across the 8 cores ([16,25000] -> [128,25000] = the next
    table, exactly).
  - Per-core node tensors (root input, h1..h3) live in two [128, 25000] bf16
    buffers (rows = channels, zero-padded; SBUF charges 128 partitions either
    way), so every matmul is a plain full-height position-(0,0) matmul.
  - segment_max pooling + final linear run on host from the [8, 25000] bf16
    per-core layer-3 output (tiny download).
Any device failure falls back to a pure-numpy reference implementation.
"""
import sys
import time
import types

import numpy as np

sys.path.insert(0, "/opt/trn_rl_repo")

N = 200_000
E = 6_400_000
G = 512
NC = 8            # cores (dst shards)
NPC = 25_000      # nodes per core
NSTR = 8          # streams = src blocks
NBS = 25_000      # nodes per src block
NWIN = 50         # chunks per core per layer
WIN = 500         # dst nodes per chunk window (x4B fits one PSUM bank)
BCH = 512         # boundary slots per chunk (501 padded to mult 16)
GRP = 3_125       # nodes per fold group (= NPC / 8)

CO = (32, 16, 8)          # layer output channels
CRAW = (16, 32, 16)       # root-term contraction (padded) channels


# ---------------------------------------------------------------- host prep

def _build_streams(src, dst):
    """eidx [NC, NWIN, 128, CH//16] u16, bidx [NC, NWIN, 128, BCH//16] u16."""
    k = dst // NPC
    j = src // NBS
    dl = dst % NPC
    c = dl // WIN
    tloc = (src % NBS).astype(np.uint16)

    key = ((k * NSTR + j) * NWIN + c) * NPC + dl
    order = np.argsort(key, kind="stable")
    tloc_s = tloc[order]
    grp = (key // NPC)[order]
    dl_s = dl[order]

    ngrp = NC * NSTR * NWIN
    cnt = np.bincount(grp, minlength=ngrp)
    CH = int(16 * ((cnt.max() + 1 + 15) // 16))
    starts = np.zeros(ngrp + 1, np.int64)
    np.cumsum(cnt, out=starts[1:])
    pos = np.arange(len(src)) - starts[grp] + 1

    eidx = np.zeros((ngrp, CH), np.uint16)
    eidx[grp, pos] = tloc_s

    node_in_win = dl_s % WIN
    nkey = grp * WIN + node_in_win
    ncnt = np.bincount(nkey, minlength=ngrp * WIN).reshape(ngrp, WIN)
    P = np.zeros((ngrp, BCH), np.int64)
    np.cumsum(ncnt, axis=1, out=P[:, 1:WIN + 1])
    P[:, WIN + 1:] = P[:, WIN:WIN + 1]
    assert P.max() < CH
    bidx = P.astype(np.uint16)

    def wrap(a, L):
        a = a.reshape(NC, NSTR, NWIN, L // 16, 16)
        a = a.transpose(0, 2, 1, 4, 3).reshape(NC, NWIN, 128, L // 16)
        return np.ascontiguousarray(a)

    return wrap(eidx, CH), wrap(bidx, BCH), CH


def _make_xtab(z16, dtype):
    """z16 [N, 16] -> [128, NBS]: row 16j+c = ch c of nodes [NBS*j, NBS*(j+1))."""
    t = np.zeros((128, NBS), np.float32)
    for j in range(NSTR):
        t[16 * j:16 * j + 16, :] = z16[j * NBS:(j + 1) * NBS, :].T
    return t.astype(dtype)


def _pad128(w, cols=None):
    """w [rows_real, co] -> [128, cols or co] zero-padded."""
    cols = w.shape[1] if cols is None else cols
    out = np.zeros((128, cols), np.float32)
    out[:w.shape[0], :w.shape[1]] = w
    return out


def _np_reference(x, src, dst, batch, Ws):
    (W1r, b1, W1o, W2r, b2, W2o, W3r, b3, W3o, Wl, bl) = Ws
    def conv(h, Wr, b, Wo):
        agg = np.zeros((N, h.shape[1]), np.float32)
        np.add.at(agg, dst, h[src])
        return np.maximum(agg @ Wr.T + b + h @ Wo.T, 0.0)
    h = conv(x, W1r, b1, W1o)
    h = conv(h, W2r, b2, W2o)
    h = conv(h, W3r, b3, W3o)
    pooled = np.full((G, 8), -np.inf, np.float32)
    np.maximum.at(pooled, batch, h)
    return (pooled @ Wl.T + bl)[:, 0]


# ---------------------------------------------------------------- device

def _build_device(CH):
    import concourse.bass as bass
    import concourse.mybir as mybir
    import concourse.tile as tile
    import kutil_inline

    kutil_inline.patch_tile()
    f32, f32r, bf16, u16 = (mybir.dt.float32, mybir.dt.float32r,
                            mybir.dt.bfloat16, mybir.dt.uint16)
    nc = bass.Bass("TRN2", num_devices=NC)

    xtabB_h = nc.dram_tensor("xtabB", [128, NBS], bf16, kind="ExternalInput")
    xroot_h = nc.dram_tensor("xroot", [16, NPC], bf16, kind="ExternalInput")
    eidx_h = nc.dram_tensor("eidx", [NWIN, 128, CH // 16], u16,
                            kind="ExternalInput")
    bidx_h = nc.dram_tensor("bidx", [NWIN, 128, BCH // 16], u16,
                            kind="ExternalInput")
    tw_h = [nc.dram_tensor(f"tw{l}", [128, CO[l]], f32r, kind="ExternalInput")
            for l in range(3)]
    wroot_h = [nc.dram_tensor(f"wroot{l}", [128, CO[l]], bf16,
                              kind="ExternalInput") for l in range(3)]
    wz_h = [nc.dram_tensor(f"wz{l}", [128, 16], bf16, kind="ExternalInput")
            for l in range(2)]
    bias_h = [nc.dram_tensor(f"bias{l}", [CO[l], 1], f32, kind="ExternalInput")
              for l in range(3)]
    h3_h = nc.dram_tensor("h3", [8, NPC], bf16, kind="ExternalOutput")

    with tile.TileContext(nc) as tc:
        with tc.tile_pool(name="p", bufs=1) as pool, \
             tc.tile_pool(name="ps", bufs=3, space="PSUM") as psp, \
             tc.tile_pool(name="dram", bufs=1, space="DRAM") as dram:

            tw = []
            for l in range(3):
                t = pool.tile([128, CO[l]], f32r, name=f"tw{l}", tag=f"tw{l}")
                nc.sync.dma_start(t[:], tw_h[l][:, :])
                tw.append(t)
            wroot = []
            for l in range(3):
                t = pool.tile([128, CO[l]], bf16, name=f"wroot{l}",
                              tag=f"wroot{l}")
                nc.sync.dma_start(t[:], wroot_h[l][:, :])
                wroot.append(t)
            wz = []
            for l in range(2):
                t = pool.tile([128, 16], bf16, name=f"wz{l}", tag=f"wz{l}")
                nc.sync.dma_start(t[:], wz_h[l][:, :])
                wz.append(t)
            bias = []
            for l in range(3):
                b = pool.tile([CO[l], 1], f32, name=f"bias{l}", tag=f"bias{l}")
                nc.sync.dma_start(b[:], bias_h[l][:, :])
                bias.append(b)

            # two channel-major node buffers (rows = channels, zero padded)
            bufA = pool.tile([128, NPC], bf16)   # xroot (L1) then h2 (L2/L3)
            bufB = pool.tile([128, NPC], bf16)   # h1 (L1/L2) then h3 (L3)
            nc.vector.memset(bufA[:], 0.0)
            nc.vector.memset(bufB[:], 0.0)
            nc.sync.dma_start(bufA[0:16, :], xroot_h[:, :])
            table = pool.tile([128, NBS], bf16)

            zb = dram.tile([16, NBS], bf16)
            ccout = [dram.tile([128, NBS], bf16, addr_space="Shared",
                               name=f"ccout{i}", tag=f"ccout{i}")
                     for i in range(2)]

            zsrc = [bufA, bufB, bufA]      # root input per layer
            hdst = [(bufB, 32), (bufA, 16), (bufB, 8)]  # h output per layer

            for l in range(3):
                if l == 0:
                    nc.sync.dma_start(table[:], xtabB_h[:, :])
                else:
                    nc.sync.dma_start(table[:], ccout[l - 1][:, :])
                for c in range(NWIN):
                    cw = c * WIN
                    ei = pool.tile([128, CH // 16], u16, tag="ei", bufs=3)
                    nc.sync.dma_start(ei[:], eidx_h[c, :, :])
                    gout = pool.tile([128, CH], bf16, tag="gout", bufs=2)
                    for off in range(0, CH, 1024):
                        n = min(1024, CH - off)
                        nc.gpsimd.indirect_copy(
                            gout[:, off:off + n], table[:],
                            ei[:, off // 16:(off + n) // 16],
                            i_know_ap_gather_is_preferred=True)
                    sout = pool.tile([128, CH], f32, tag="sout", bufs=2)
                    nc.vector.tensor_tensor_scan(
                        sout[:], gout[:], gout[:], 0.0,
                        op0=mybir.AluOpType.add, op1=mybir.AluOpType.bypass)
                    bi = pool.tile([128, BCH // 16], u16, tag="bi", bufs=3)
                    nc.sync.dma_start(bi[:], bidx_h[c, :, :])
                    bout = pool.tile([128, BCH], f32, tag="bout", bufs=2)
                    nc.gpsimd.indirect_copy(
                        bout[:], sout[:], bi[:],
                        i_know_ap_gather_is_preferred=True)
                    dout = pool.tile([128, WIN], f32r, tag="dout", bufs=2)
                    nc.vector.tensor_tensor(
                        dout[:], bout[:, 1:WIN + 1], bout[:, 0:WIN],
                        op=mybir.AluOpType.subtract)
                    ps = psp.tile([CO[l], WIN], f32, tag="ps")
                    nc.tensor.matmul(
                        ps[:], tw[l][:], dout[:],
                        start=True, stop=False)
                    nc.tensor.matmul(
                        ps[:], wroot[l][:], zsrc[l][:, cw:cw + WIN],
                        start=False, stop=True)
                    htile, nch = hdst[l]
                    nc.scalar.activation(
                        htile[0:nch, cw:cw + WIN], ps[:],
                        mybir.ActivationFunctionType.Relu, bias=bias[l][:])
                if l < 2:
                    hp = bufB if l == 0 else bufA
                    for c in range(NWIN):
                        cw = c * WIN
                        psz = psp.tile([16, WIN], f32, tag="ps")
                        nc.tensor.matmul(psz[:], wz[l][:],
                                         hp[:, cw:cw + WIN],
                                         start=True, stop=True)
                        zst = pool.tile([16, WIN], bf16, tag="zst", bufs=3)
                        nc.scalar.activation(
                            zst[:], psz[:],
                            mybir.ActivationFunctionType.Copy)
                        nc.sync.dma_start(zb[:, cw:cw + WIN], zst[:])
                    nc.gpsimd.collective_compute(
                        "AllGather", mybir.AluOpType.bypass,
                        replica_groups=[list(range(NC))],
                        ins=[zb[:]], outs=[ccout[l][:]])
            nc.sync.dma_start(h3_h[:, :], bufB[0:8, :])
    return nc


# ---------------------------------------------------------------- kernel

def kernel(**inputs):
    kernel.measure_repeat = getattr(kernel, "measure_repeat", False)
    x = np.asarray(inputs["x"], np.float32)
    ei = np.asarray(inputs["edge_index"])
    batch = np.asarray(inputs["batch"]).astype(np.int64)
    src = ei[0].astype(np.int64)
    dst = ei[1].astype(np.int64)
    Ws = tuple(np.asarray(inputs[n], np.float32) for n in
               ("W1_rel", "b1", "W1_root", "W2_rel", "b2", "W2_root",
                "W3_rel", "b3", "W3_root", "W_lin", "b_lin"))
    (W1r, b1, W1o, W2r, b2, W2o, W3r, b3, W3o, Wl, bl) = Ws
    try:
        import ml_dtypes
        from concourse.bass_utils import run_bass_kernel_spmd

        t0 = time.time()
        eidx, bidx, CH = _build_streams(src, dst)
        z16 = np.zeros((N, 16), np.float32)
        z16[:, :3] = x
        xtabB = _make_xtab(z16, ml_dtypes.bfloat16)
        tw = [np.zeros((128, CO[l]), np.float32) for l in range(3)]
        for j in range(NSTR):
            tw[0][16 * j:16 * j + 3, :] = W1r.T
            tw[1][16 * j:16 * j + 16, :] = np.eye(16, dtype=np.float32)
            tw[2][16 * j:16 * j + 8, :] = np.eye(8, dtype=np.float32)
        prep_s = time.time() - t0

        bf = ml_dtypes.bfloat16
        common = {
            "xtabB": xtabB,
            "tw0": tw[0], "tw1": tw[1], "tw2": tw[2],
            "wroot0": _pad128(W1o.T).astype(bf),
            "wroot1": _pad128(W2o.T).astype(bf),
            "wroot2": _pad128(W3o.T).astype(bf),
            "wz0": _pad128(W2r.T, 16).astype(bf),
            "wz1": _pad128(W3r.T, 16).astype(bf),
            "bias0": b1[:, None].astype(np.float32),
            "bias1": b2[:, None].astype(np.float32),
            "bias2": b3[:, None].astype(np.float32),
        }
        in_maps = []
        for k in range(NC):
            xr = np.zeros((16, NPC), np.float32)
            xr[:3, :] = x[k * NPC:(k + 1) * NPC, :].T
            in_maps.append(dict(
                common, xroot=xr.astype(bf),
                eidx=eidx[k], bidx=bidx[k]))

        t0 = time.time()
        nc = _build_device(CH)
        build_s = time.time() - t0
        t0 = time.time()
        res = run_bass_kernel_spmd(nc, in_maps, core_ids=list(range(NC)))
        exec_s = time.time() - t0
        rep_s = -1.0
        if kernel.measure_repeat:
            t0 = time.time()
            res = run_bass_kernel_spmd(nc, in_maps, core_ids=list(range(NC)))
            rep_s = time.time() - t0
        kernel.last_hw_s = rep_s if rep_s >= 0 else exec_s
        kernel.last_info = (prep_s, build_s, exec_s, rep_s)

        h3 = np.concatenate(
            [np.asarray(res.results[k]["h3"]).astype(np.float32).T
             for k in range(NC)], axis=0)
        bounds = np.searchsorted(batch, np.arange(G))
        pooled = np.maximum.reduceat(h3, bounds, axis=0)
        empty = bounds == np.append(bounds[1:], N)
        pooled[empty] = 0.0
        out = (pooled @ Wl.T + bl)[:, 0].astype(np.float32)
        return out
    except Exception as e:  # pragma: no cover - device fallback
        import traceback
        traceback.print_exc()
        print(f"[kernel] device path failed ({type(e).__name__}: {e}); "
              f"falling back to numpy", file=sys.stderr)
        kernel.last_hw_s = -1.0
        return _np_reference(x, src, dst, batch, Ws).astype(np.float32)


# inline copy of the tile patch so kernel.py is self-contained
kutil_inline = types.ModuleType("kutil_inline")
_KUTIL_SRC = '''
import sys
sys.path.insert(0, "/opt/trn_rl_repo")
import bass_rust
import concourse.mybir as mybir
import concourse.tile as tilemod
from concourse.vector_clock import ScopedClock

MAX_WAITS = 1
_patched = False


def _split_waits(nc, ordered):
    for bb_name, insts in ordered.items():
        new_list = []
        for inst in insts:
            si = inst.sync_info
            waits = list(si.on_wait) if si is not None and si.on_wait else []
            if len(waits) > MAX_WAITS:
                keep = waits[-MAX_WAITS:]
                extra = waits[:-MAX_WAITS]
                for w in extra:
                    nop = bass_rust.InstNoOp(
                        name=f"I-{nc.next_id()}-waitnop", ins=[], outs=[]
                    )
                    nop.engine = inst.engine
                    nop.sync_info = mybir.SyncInfo(on_wait=[w], on_update=[])
                    nc.register_instruction(nop, overwrite=True)
                    new_list.append(nop)
                inst.sync_info = mybir.SyncInfo(
                    on_wait=keep,
                    on_update=list(si.on_update) if si.on_update else [],
                )
            new_list.append(inst)
        insts[:] = new_list
    return ordered


def patch_tile(verbose=False):
    global _patched
    if _patched:
        return
    _patched = True
    orig_lower = tilemod.TileContext._lower_ordered_insts

    def _lower_ordered_insts(self, ordered):
        _split_waits(self.nc, ordered)
        return orig_lower(self, ordered)

    tilemod.TileContext._lower_ordered_insts = _lower_ordered_insts

    def _drain_and_barrier(self, tick_clock, wait_clock):
        drain_inst = self.nc.sync.drain()
        wait_clock.add_sem_waits(
            drain_inst.ins, ScopedClock({None: tick_clock.global_clock})
        )
        si = drain_inst.ins.sync_info
        waits = list(si.on_wait) if si is not None and si.on_wait else []
        if len(waits) > MAX_WAITS:
            drain_inst.ins.sync_info = mybir.SyncInfo(
                on_wait=waits[:MAX_WAITS],
                on_update=list(si.on_update) if si.on_update else [],
            )
            for i in range(MAX_WAITS, len(waits), MAX_WAITS):
                d2 = self.nc.sync.drain()
                d2.ins.sync_info = mybir.SyncInfo(
                    on_wait=waits[i : i + MAX_WAITS], on_update=[]
                )
        self.nc.all_engine_barrier()
        assert self.sems is not None
        popped = self.nc._tile_sem_poison_stack.pop()
        assert popped is self._sem_poison
        self.nc.clear_and_free_semaphores(list(self.sems.allocated().values()))
        self.nc.all_engine_barrier()

    tilemod.TileContext._drain_and_barrier = _drain_and_barrier
'''
exec(_KUTIL_SRC, kutil_inline.__dict__)
sys.modules["kutil_inline"] = kutil_inline


# revision 29
# speedup vs baseline: 111.1949x; 1.1497x over previous
"""GraphConv x3 + segment_max pooling on Trainium2 (8 NeuronCores, one launch).

Algorithm (per core, dst-sharded 25000 nodes):
  - 8 gather streams = src blocks of 25000 nodes; partition group j holds the
    feature table of block j ([16ch x 25000] bf16) -> no table replication.
  - Per layer, 50 chunks over the core's dst nodes (500-node windows):
    Pool:  indirect_copy gathers each stream's edges (sorted by dst) from its
           table group (<=1024 indices per instruction); a second
           indirect_copy picks node-boundary positions out of the running sum.
    DVE:   tensor_tensor_scan computes the fp32 running sum of the gathered
           bf16 edge stream; a shifted subtract turns boundary samples into
           per-(node, stream) partial sums.
    PE:    one matmul folds the 8 stream partials and the W_rel transform
           (stationary [128, Co]); a second accumulates the root term.
    ACT:   relu(psum + bias) evacuation.
  - Layers 2/3 aggregate pre-transformed features (h @ W_rel.T, <=16 ch), so
    the next layer's table is built by a small chunked matmul, bounced to HBM
    and AllGathered across the 8 cores ([16,25000] -> [128,25000] = the next
    table, exactly).
  - Per-core node tensors (root input, h1..h3) live in two [128, 25000] bf16
    buffers (rows = channels, zero-padded; SBUF charges 128 partitions either
    way), so every matmul is a plain full-height position-(0,0) matmul.
  - segment_max pooling + final linear run on host from the [8, 25000] bf16
    per-core layer-3 output (tiny download).
Any device failure falls back to a pure-numpy reference implementation.
"""
import sys
import time
import types

import numpy as np

sys.path.insert(0, "/opt/trn_rl_repo")

N = 200_000
E = 6_400_000
G = 512
NC = 8            # cores (dst shards)
NPC = 25_000      # nodes per core
NSTR = 8          # streams = src blocks
NBS = 25_000      # nodes per src block
NWIN = 50         # chunks per core per layer
WIN = 500         # dst nodes per chunk window (x4B fits one PSUM bank)
BCH = 512         # boundary slots per chunk (501 padded to mult 16)
GRP = 3_125       # nodes per fold group (= NPC / 8)

CO = (32, 16, 8)          # layer output channels
CRAW = (16, 32, 16)       # root-term contraction (padded) channels


# ---------------------------------------------------------------- host prep

def _build_streams(src, dst):
    """eidx [NC, NWIN, 128, CH//16] u16, bidx [NC, NWIN, 128, BCH//16] u16."""
    k = dst // NPC
    j = src // NBS
    dl = dst % NPC
    c = dl // WIN
    tloc = (src % NBS).astype(np.uint16)

    key = ((k * NSTR + j) * NWIN + c) * NPC + dl
    order = np.argsort(key, kind="stable")
    tloc_s = tloc[order]
    grp = (key // NPC)[order]
    dl_s = dl[order]

    ngrp = NC * NSTR * NWIN
    cnt = np.bincount(grp, minlength=ngrp)
    CH = int(16 * ((cnt.max() + 1 + 15) // 16))
    starts = np.zeros(ngrp + 1, np.int64)
    np.cumsum(cnt, out=starts[1:])
    pos = np.arange(len(src)) - starts[grp] + 1

    eidx = np.zeros((ngrp, CH), np.uint16)
    eidx[grp, pos] = tloc_s

    node_in_win = dl_s % WIN
    nkey = grp * WIN + node_in_win
    ncnt = np.bincount(nkey, minlength=ngrp * WIN).reshape(ngrp, WIN)
    P = np.zeros((ngrp, BCH), np.int64)
    np.cumsum(ncnt, axis=1, out=P[:, 1:WIN + 1])
    P[:, WIN + 1:] = P[:, WIN:WIN + 1]
    assert P.max() < CH
    bidx = P.astype(np.uint16)

    def wrap(a, L):
        a = a.reshape(NC, NSTR, NWIN, L // 16, 16)
        a = a.transpose(0, 2, 1, 4, 3).reshape(NC, NWIN, 128, L // 16)
        return np.ascontiguousarray(a)

    return wrap(eidx, CH), wrap(bidx, BCH), CH


def _make_xtab(z16, dtype):
    """z16 [N, 16] -> [128, NBS]: row 16j+c = ch c of nodes [NBS*j, NBS*(j+1))."""
    t = np.zeros((128, NBS), np.float32)
    for j in range(NSTR):
        t[16 * j:16 * j + 16, :] = z16[j * NBS:(j + 1) * NBS, :].T
    return t.astype(dtype)


def _pad128(w, cols=None):
    """w [rows_real, co] -> [128, cols or co] zero-padded."""
    cols = w.shape[1] if cols is None else cols
    out = np.zeros((128, cols), np.float32)
    out[:w.shape[0], :w.shape[1]] = w
    return out


def _np_reference(x, src, dst, batch, Ws):
    (W1r, b1, W1o, W2r, b2, W2o, W3r, b3, W3o, Wl, bl) = Ws
    def conv(h, Wr, b, Wo):
        agg = np.zeros((N, h.shape[1]), np.float32)
        np.add.at(agg, dst, h[src])
        return np.maximum(agg @ Wr.T + b + h @ Wo.T, 0.0)
    h = conv(x, W1r, b1, W1o)
    h = conv(h, W2r, b2, W2o)
    h = conv(h, W3r, b3, W3o)
    pooled = np.full((G, 8), -np.inf, np.float32)
    np.maximum.at(pooled, batch, h)
    return (pooled @ Wl.T + bl)[:, 0]


# ---------------------------------------------------------------- device

def _build_device(CH):
    import concourse.bass as bass
    import concourse.mybir as mybir
    import concourse.tile as tile
    import kutil_inline

    kutil_inline.patch_tile()
    f32, f32r, bf16, u16 = (mybir.dt.float32, mybir.dt.float32r,
                            mybir.dt.bfloat16, mybir.dt.uint16)
    nc = bass.Bass("TRN2", num_devices=NC)

    xtabB_h = nc.dram_tensor("xtabB", [128, NBS], bf16, kind="ExternalInput")
    xroot_h = nc.dram_tensor("xroot", [16, NPC], bf16, kind="ExternalInput")
    eidx_h = nc.dram_tensor("eidx", [NWIN, 128, CH // 16], u16,
                            kind="ExternalInput")
    bidx_h = nc.dram_tensor("bidx", [NWIN, 128, BCH // 16], u16,
                            kind="ExternalInput")
    tw_h = [nc.dram_tensor(f"tw{l}", [128, CO[l]], f32r, kind="ExternalInput")
            for l in range(3)]
    wroot_h = [nc.dram_tensor(f"wroot{l}", [128, CO[l]], bf16,
                              kind="ExternalInput") for l in range(3)]
    wz_h = [nc.dram_tensor(f"wz{l}", [128, 16], bf16, kind="ExternalInput")
            for l in range(2)]
    bias_h = [nc.dram_tensor(f"bias{l}", [CO[l], 1], f32, kind="ExternalInput")
              for l in range(3)]
    h3_h = nc.dram_tensor("h3", [8, NPC], bf16, kind="ExternalOutput")

    with tile.TileContext(nc) as tc:
        with tc.tile_pool(name="p", bufs=1) as pool, \
             tc.tile_pool(name="ps", bufs=3, space="PSUM") as psp, \
             tc.tile_pool(name="dram", bufs=1, space="DRAM") as dram:

            tw = []
            for l in range(3):
                t = pool.tile([128, CO[l]], f32r, name=f"tw{l}", tag=f"tw{l}")
                nc.sync.dma_start(t[:], tw_h[l][:, :])
                tw.append(t)
            wroot = []
            for l in range(3):
                t = pool.tile([128, CO[l]], bf16, name=f"wroot{l}",
                              tag=f"wroot{l}")
                nc.sync.dma_start(t[:], wroot_h[l][:, :])
                wroot.append(t)
            wz = []
            for l in range(2):
                t = pool.tile([128, 16], bf16, name=f"wz{l}", tag=f"wz{l}")
                nc.sync.dma_start(t[:], wz_h[l][:, :])
                wz.append(t)
            bias = []
            for l in range(3):
                b = pool.tile([CO[l], 1], f32, name=f"bias{l}", tag=f"bias{l}")
                nc.sync.dma_start(b[:], bias_h[l][:, :])
                bias.append(b)

            # two channel-major node buffers (rows = channels, zero padded)
            bufA = pool.tile([128, NPC], bf16)   # xroot (L1) then h2 (L2/L3)
            bufB = pool.tile([128, NPC], bf16)   # h1 (L1/L2) then h3 (L3)
            nc.vector.memset(bufA[:], 0.0)
            nc.vector.memset(bufB[:], 0.0)
            nc.sync.dma_start(bufA[0:16, :], xroot_h[:, :])
            table = pool.tile([128, NBS], bf16)

            HNB = NBS // 2
            zb = [dram.tile([16, HNB], bf16, name=f"zb{i}", tag=f"zb{i}")
                  for i in range(2)]
            ccout = [[dram.tile([128, HNB], bf16, addr_space="Shared",
                                name=f"ccout{i}_{h}", tag=f"ccout{i}_{h}")
                      for h in range(2)] for i in range(2)]

            zsrc = [bufA, bufB, bufA]      # root input per layer
            hdst = [(bufB, 32), (bufA, 16), (bufB, 8)]  # h output per layer

            for l in range(3):
                if l == 0:
                    Q = NBS // 4
                    for s in range(4):
                        eng = nc.sync if s % 2 == 0 else nc.scalar
                        eng.dma_start(table[:, s * Q:(s + 1) * Q],
                                      xtabB_h[:, s * Q:(s + 1) * Q])
                else:
                    Q = NBS // 4
                    for s in range(4):
                        eng = nc.sync if s % 2 == 0 else nc.scalar
                        h = s // 2
                        o = (s % 2) * Q
                        eng.dma_start(
                            table[:, h * HNB + o:h * HNB + o + Q],
                            ccout[l - 1][h][:, o:o + Q])
                for c in range(NWIN):
                    cw = c * WIN
                    ei = pool.tile([128, CH // 16], u16, tag="ei", bufs=3)
                    nc.scalar.dma_start(ei[:], eidx_h[c, :, :])
                    gout = pool.tile([128, CH], bf16, tag="gout", bufs=2)
                    for off in range(0, CH, 1024):
                        n = min(1024, CH - off)
                        nc.gpsimd.indirect_copy(
                            gout[:, off:off + n], table[:],
                            ei[:, off // 16:(off + n) // 16],
                            i_know_ap_gather_is_preferred=True)
                    sout = pool.tile([128, CH], f32, tag="sout", bufs=2)
                    nc.vector.tensor_tensor_scan(
                        sout[:], gout[:], gout[:], 0.0,
                        op0=mybir.AluOpType.add, op1=mybir.AluOpType.bypass)
                    bi = pool.tile([128, BCH // 16], u16, tag="bi", bufs=3)
                    nc.scalar.dma_start(bi[:], bidx_h[c, :, :])
                    bout = pool.tile([128, BCH], f32, tag="bout", bufs=2)
                    nc.gpsimd.indirect_copy(
                        bout[:], sout[:], bi[:],
                        i_know_ap_gather_is_preferred=True)
                    dout = pool.tile([128, WIN], f32r, tag="dout", bufs=2)
                    nc.vector.tensor_tensor(
                        dout[:], bout[:, 1:WIN + 1], bout[:, 0:WIN],
                        op=mybir.AluOpType.subtract)
                    ps = psp.tile([CO[l], WIN], f32, tag="ps")
                    nc.tensor.matmul(
                        ps[:], tw[l][:], dout[:],
                        start=True, stop=False)
                    nc.tensor.matmul(
                        ps[:], wroot[l][:], zsrc[l][:, cw:cw + WIN],
                        start=False, stop=True)
                    htile, nch = hdst[l]
                    nc.scalar.activation(
                        htile[0:nch, cw:cw + WIN], ps[:],
                        mybir.ActivationFunctionType.Relu, bias=bias[l][:])
                    if l < 2:
                        hp = bufB if l == 0 else bufA
                        psz = psp.tile([16, WIN], f32, tag="ps")
                        nc.tensor.matmul(psz[:], wz[l][:],
                                         hp[:, cw:cw + WIN],
                                         start=True, stop=True)
                        zst = pool.tile([16, WIN], bf16, tag="zst", bufs=3)
                        nc.scalar.activation(
                            zst[:], psz[:],
                            mybir.ActivationFunctionType.Copy)
                        h = c // (NWIN // 2)
                        nc.sync.dma_start(
                            zb[h][:, cw - h * HNB:cw - h * HNB + WIN],
                            zst[:])
                        if c == NWIN // 2 - 1 or c == NWIN - 1:
                            nc.gpsimd.collective_compute(
                                "AllGather", mybir.AluOpType.bypass,
                                replica_groups=[list(range(NC))],
                                ins=[zb[h][:]], outs=[ccout[l][h][:]])
            nc.sync.dma_start(h3_h[:, :], bufB[0:8, :])
    return nc


# ---------------------------------------------------------------- kernel

def kernel(**inputs):
    kernel.measure_repeat = getattr(kernel, "measure_repeat", False)
    x = np.asarray(inputs["x"], np.float32)
    ei = np.asarray(inputs["edge_index"])
    batch = np.asarray(inputs["batch"]).astype(np.int64)
    src = ei[0].astype(np.int64)
    dst = ei[1].astype(np.int64)
    Ws = tuple(np.asarray(inputs[n], np.float32) for n in
               ("W1_rel", "b1", "W1_root", "W2_rel", "b2", "W2_root",
                "W3_rel", "b3", "W3_root", "W_lin", "b_lin"))
    (W1r, b1, W1o, W2r, b2, W2o, W3r, b3, W3o, Wl, bl) = Ws
    try:
        import ml_dtypes
        from concourse.bass_utils import run_bass_kernel_spmd

        t0 = time.time()
        eidx, bidx, CH = _build_streams(src, dst)
        z16 = np.zeros((N, 16), np.float32)
        z16[:, :3] = x
        xtabB = _make_xtab(z16, ml_dtypes.bfloat16)
        tw = [np.zeros((128, CO[l]), np.float32) for l in range(3)]
        for j in range(NSTR):
            tw[0][16 * j:16 * j + 3, :] = W1r.T
            tw[1][16 * j:16 * j + 16, :] = np.eye(16, dtype=np.float32)
            tw[2][16 * j:16 * j + 8, :] = np.eye(8, dtype=np.float32)
        prep_s = time.time() - t0

        bf = ml_dtypes.bfloat16
        common = {
            "xtabB": xtabB,
            "tw0": tw[0], "tw1": tw[1], "tw2": tw[2],
            "wroot0": _pad128(W1o.T).astype(bf),
            "wroot1": _pad128(W2o.T).astype(bf),
            "wroot2": _pad128(W3o.T).astype(bf),
            "wz0": _pad128(W2r.T, 16).astype(bf),
            "wz1": _pad128(W3r.T, 16).astype(bf),
            "bias0": b1[:, None].astype(np.float32),
            "bias1": b2[:, None].astype(np.float32),
            "bias2": b3[:, None].astype(np.float32),
        }
        in_maps = []
        for k in range(NC):
            xr = np.zeros((16, NPC), np.float32)
            xr[:3, :] = x[k * NPC:(k + 1) * NPC, :].T
            in_maps.append(dict(
                common, xroot=xr.astype(bf),
                eidx=eidx[k], bidx=bidx[k]))

        t0 = time.time()
        nc = _build_device(CH)
        build_s = time.time() - t0
        t0 = time.time()
        res = run_bass_kernel_spmd(nc, in_maps, core_ids=list(range(NC)))
        exec_s = time.time() - t0
        rep_s = -1.0
        if kernel.measure_repeat:
            t0 = time.time()
            res = run_bass_kernel_spmd(nc, in_maps, core_ids=list(range(NC)))
            rep_s = time.time() - t0
        kernel.last_hw_s = rep_s if rep_s >= 0 else exec_s
        kernel.last_info = (prep_s, build_s, exec_s, rep_s)

        h3 = np.concatenate(
            [np.asarray(res.results[k]["h3"]).astype(np.float32).T
             for k in range(NC)], axis=0)
        bounds = np.searchsorted(batch, np.arange(G))
        pooled = np.maximum.reduceat(h3, bounds, axis=0)
        empty = bounds == np.append(bounds[1:], N)
        pooled[empty] = 0.0
        out = (pooled @ Wl.T + bl)[:, 0].astype(np.float32)
        return out
    except Exception as e:  # pragma: no cover - device fallback
        import traceback
        traceback.print_exc()
        print(f"[kernel] device path failed ({type(e).__name__}: {e}); "
              f"falling back to numpy", file=sys.stderr)
        kernel.last_hw_s = -1.0
        return _np_reference(x, src, dst, batch, Ws).astype(np.float32)


# inline copy of the tile patch so kernel.py is self-contained
kutil_inline = types.ModuleType("kutil_inline")
_KUTIL_SRC = '''
import sys
sys.path.insert(0, "/opt/trn_rl_repo")
import bass_rust
import concourse.mybir as mybir
import concourse.tile as tilemod
from concourse.vector_clock import ScopedClock

MAX_WAITS = 1
_patched = False


def _split_waits(nc, ordered):
    for bb_name, insts in ordered.items():
        new_list = []
        for inst in insts:
            si = inst.sync_info
            waits = list(si.on_wait) if si is not None and si.on_wait else []
            if len(waits) > MAX_WAITS:
                keep = waits[-MAX_WAITS:]
                extra = waits[:-MAX_WAITS]
                for w in extra:
                    nop = bass_rust.InstNoOp(
                        name=f"I-{nc.next_id()}-waitnop", ins=[], outs=[]
                    )
                    nop.engine = inst.engine
                    nop.sync_info = mybir.SyncInfo(on_wait=[w], on_update=[])
                    nc.register_instruction(nop, overwrite=True)
                    new_list.append(nop)
                inst.sync_info = mybir.SyncInfo(
                    on_wait=keep,
                    on_update=list(si.on_update) if si.on_update else [],
                )
            new_list.append(inst)
        insts[:] = new_list
    return ordered


def patch_tile(verbose=False):
    global _patched
    if _patched:
        return
    _patched = True
    orig_lower = tilemod.TileContext._lower_ordered_insts

    def _lower_ordered_insts(self, ordered):
        _split_waits(self.nc, ordered)
        return orig_lower(self, ordered)

    tilemod.TileContext._lower_ordered_insts = _lower_ordered_insts

    def _drain_and_barrier(self, tick_clock, wait_clock):
        drain_inst = self.nc.sync.drain()
        wait_clock.add_sem_waits(
            drain_inst.ins, ScopedClock({None: tick_clock.global_clock})
        )
        si = drain_inst.ins.sync_info
        waits = list(si.on_wait) if si is not None and si.on_wait else []
        if len(waits) > MAX_WAITS:
            drain_inst.ins.sync_info = mybir.SyncInfo(
                on_wait=waits[:MAX_WAITS],
                on_update=list(si.on_update) if si.on_update else [],
            )
            for i in range(MAX_WAITS, len(waits), MAX_WAITS):
                d2 = self.nc.sync.drain()
                d2.ins.sync_info = mybir.SyncInfo(
                    on_wait=waits[i : i + MAX_WAITS], on_update=[]
                )
        self.nc.all_engine_barrier()
        assert self.sems is not None
        popped = self.nc._tile_sem_poison_stack.pop()
        assert popped is self._sem_poison
        self.nc.clear_and_free_semaphores(list(self.sems.allocated().values()))
        self.nc.all_engine_barrier()

    tilemod.TileContext._drain_and_barrier = _drain_and_barrier
'''
exec(_KUTIL_SRC, kutil_inline.__dict__)
sys.modules["kutil_inline"] = kutil_inline
